# revision 71
# baseline (speedup 1.0000x reference)
"""Self-contained Trainium2 kernel for nn_DynamicConv2D (moe_routing).

Contract: kernel(**inputs) takes FULL unsharded inputs (numpy), returns the
FULL output [32, 64, 64, 128] float32. Internally shards batch across 8
NeuronCores (4 samples each), runs a Bass/Tile kernel via
run_bass_kernel_spmd, and gathers.

Device-side work per sample:
  pool  = sum(x) over H,W            (scalar-engine Identity w/ accum_out;
                                      1/4096 folded into R on host)
  att   = softmax(relu(pool@R')@A')  (tiny PE matmuls + ACT relu/exp + DVE recip)
  wmix  = sum_k att[k] * bank[k]     (fused DVE scalar_tensor_tensor MACs, fp16)
  conv  = 9-tap shifted fp16 matmuls accumulated in PSUM, per 512-pos chunk
  out   = Relu(conv + beta)          (ACT epilogue, per-partition bias;
                                      BN scale folded into bank/bias on host)

Layout: x is host-transposed to channel-major [C, H, W], zero-padded to
[C, 66, 66], and cast to fp16 so all 9 conv taps are plain access-pattern
offsets; output is produced channel-major [F, H*W] f32 and host-transposed
back to NHWC. Expert bank is BN-folded, fp16, replicated per core.
"""

import os
import sys

if "/opt/trn_rl_repo" not in sys.path:
    sys.path.insert(0, "/opt/trn_rl_repo")
# The kernel executes through the axon PJRT backend; make sure jax can see it
# if the caller's environment doesn't pin a platform.
if not os.environ.get("JAX_PLATFORMS"):
    os.environ["JAX_PLATFORMS"] = "axon"

import numpy as np

import concourse.bacc as bacc
import concourse.tile as tile
from concourse import mybir
from concourse.bass_utils import run_bass_kernel_spmd
from concourse.tile_rust import add_dep_helper


def _ensure_ntff_hook():
    """run_bass_kernel_spmd(trace=True) under axon needs antenv.axon_hooks,
    which this image's antenv package lacks. Register an equivalent module
    (ctypes into libaxon_pjrt.so) so profiled runs work."""
    try:
        from antenv import axon_hooks  # noqa: F401
        return
    except ImportError:
        pass
    import contextlib
    import ctypes
    import os
    import types

    so_path = os.environ.get("AXON_PJRT_SO", "/opt/axon/libaxon_pjrt.so")
    mod = types.ModuleType("antenv.axon_hooks")
    state = {"hook": None}

    def _make_hook():
        if not os.path.exists(so_path):
            return None
        lib = ctypes.CDLL(so_path)
        if not hasattr(lib, "axon_start_nrt_profile"):
            return None
        lib.axon_start_nrt_profile.argtypes = [
            ctypes.POINTER(ctypes.c_int64), ctypes.c_size_t]
        lib.axon_start_nrt_profile.restype = ctypes.c_int64
        lib.axon_stop_nrt_profile.argtypes = [ctypes.c_char_p]
        lib.axon_stop_nrt_profile.restype = ctypes.c_int64

        @contextlib.contextmanager
        def _hook(output_dir, device_ids):
            import jax
            jax.devices()
            if device_ids:
                ids = (ctypes.c_int64 * len(device_ids))(*device_ids)
                rc = lib.axon_start_nrt_profile(ids, len(device_ids))
            else:
                rc = lib.axon_start_nrt_profile(None, 0)
            if rc != 0:
                raise RuntimeError(f"axon_start_nrt_profile rc={rc}")
            try:
                yield
            finally:
                n = lib.axon_stop_nrt_profile(str(output_dir).encode())
                if n < 0:
                    raise RuntimeError(f"axon_stop_nrt_profile rc={n}")

        return _hook

    def get_axon_ntff_profile_hook():
        if state["hook"] is None:
            state["hook"] = _make_hook()
        return state["hook"]

    def set_axon_ntff_profile_hook(hook):
        state["hook"] = hook

    mod.get_axon_ntff_profile_hook = get_axon_ntff_profile_hook
    mod.set_axon_ntff_profile_hook = set_axon_ntff_profile_hook
    sys.modules["antenv.axon_hooks"] = mod
    try:
        import antenv
        antenv.axon_hooks = mod
    except ImportError:
        pass


F32 = mybir.dt.float32
F16 = mybir.dt.float16
AF = mybir.ActivationFunctionType
ALU = mybir.AluOpType

B, H, W, C = 32, 64, 64, 128
NCORES = 8
BPC = B // NCORES  # samples per core
HP, WP = H + 2, W + 2  # zero-padded
NPAD = HP * WP  # 4356
NPOS = H * W  # 4096
K = 4  # experts
NF = 128  # output filters
TAPS = 9
ROWS_PER_CHUNK = 8  # 8 image rows * 64 cols = 512 positions per PSUM chunk
NCHUNK = H // ROWS_PER_CHUNK
HALF = NPAD // 2
QTR = NPAD // 4  # pool reduce quarter / input DMA piece size
# hot consts: [128, 8] — needed on the routing critical path, tiny DMA first
# cold consts: [128, 129] — only needed by the (off-critical-path) beta calc
HOT_COLS = 8
COLD_COLS = 129

# tunables
WARM1, WARM2 = 26, 55  # fine-grained fp16 warm-up MMs before/within chain 0


class _Consts:
    """AP views into the packed constant SBUF tiles."""

    def __init__(self, hot, cold, ones1):
        self.red = hot[:, 0:4]         # reduction_kernel / 4096  [128, 4]
        self.attk = hot[0:4, 4:8]      # attention_kernel / 30    [4, 4]
        self.biasw = cold[0:4, 0:128]  # bias * inv               [4, 128]
        self.c1 = cold[:, 128:129]     # bn_bias - bn_mean*inv    [128, 1]
        self.ones1 = ones1             # ones (memset on device)  [1, 128]


def _pack_hot(red, attk):
    hot = np.zeros((128, HOT_COLS), dtype=np.float32)
    hot[:, 0:4] = red
    hot[0:4, 4:8] = attk
    return hot


def _pack_cold(biasw, c1):
    cold = np.zeros((128, COLD_COLS), dtype=np.float32)
    cold[0:4, 0:128] = biasw
    cold[:, 128] = c1
    return cold


def _emit_pool(nc, b, sb, xt_sb, trash, act_only=False):
    """Part A: the two pool half-reduces (ACT + DVE). Emitted ahead of the
    previous sample's conv chunks so the ACT reduce precedes its epilogues
    in the scalar-engine queue. act_only puts both halves on the scalar
    engine — used for the LAST sample so its reduce needs no DVE slot and
    no ordering against the previous sample's mixing chain."""
    pq = [sb.tile([C, 1], F32, tag="poolh", name=f"pool{b}h{i}")
          for i in range(2)]
    ia = nc.scalar.activation(trash[:], xt_sb[:, :HALF], AF.Identity,
                              accum_out=pq[0][:])
    if act_only:
        ib = nc.scalar.activation(trash[:], xt_sb[:, HALF:], AF.Identity,
                                  accum_out=pq[1][:])
    else:
        ib = nc.vector.tensor_reduce(pq[1][:], xt_sb[:, HALF:],
                                     axis=mybir.AxisListType.X, op=ALU.add)
    return {"pool_a": ia, "pool_b": ib, "pq": pq, "act_only": act_only}


def _emit_chain(nc, b, sb, ps, xt_sb, cc, wk_sb, wm_sb, beta_sb, invs_sb,
                pool):
    """Routing chain for sample b.

    Critical path (6 cross-engine hops):
      pool quarters -> pr(PE) -> relu(ACT) -> lg_row(PE) -> exp(ACT, with
      softmax denominator free via accum_out) -> att broadcast(PE) ->
      copy(DVE) -> mixing MACs(DVE).

    The softmax is left UNNORMALIZED here — mixing uses raw exp weights and
    the 1/sum lands in the epilogue's per-partition activation scale
    (invs_sb), along with the matching bias correction (beta_sb). That whole
    normalization branch runs off the critical path, in parallel with the
    mixing.
    """
    # Global pool via four quarter-reduces, two on the scalar engine
    # (Identity with running-sum accumulator) and two on DVE, all running
    # concurrently as the input quarters land (padding zeros don't change
    # the sum).
    pq = [sb.tile([C, 1], F32, tag="poolh", name=f"pool{b}h{i}")
          for i in range(2)]
    ia = nc.scalar.activation(trash[:], xt_sb[:, :HALF], AF.Identity,
                              accum_out=pq[0][:])
    ib = nc.scalar.activation(trash[:], xt_sb[:, HALF:], AF.Identity,
                              accum_out=pq[1][:])

    # pool_red.T = (R/4096).T @ (pq0 + pq1) via two accumulating matmuls
    pr_ps = ps.tile([K, 1], F32, tag="tiny")
    for i in range(2):
        nc.tensor.matmul(pr_ps[:], cc.red, pq[i][:], start=(i == 0),
                         stop=(i == 1))
    prelu_sb = sb.tile([K, 1], F32, tag="prelu")
    nc.scalar.activation(prelu_sb[:], pr_ps[:], AF.Relu)

    # logits as a ROW: lg_row = pool_red @ (A/30)  -> [1, 4]
    lgr_ps = ps.tile([1, K], F32, tag="tiny")
    nc.tensor.matmul(lgr_ps[:], prelu_sb[:], cc.attk, start=True, stop=True)
    # e_row = exp(lg_row); the softmax denominator comes free via accum_out
    er_sb = sb.tile([1, K], F32, tag="erow")
    s_sb = sb.tile([1, 1], F32, tag="ssum")
    nc.scalar.activation(er_sb[:], lgr_ps[:], AF.Exp, accum_out=s_sb[:])
    # broadcast raw exp weights to all 128 partitions
    ab_ps = ps.tile([C, K], F32, tag="tiny")
    nc.tensor.matmul(ab_ps[:], cc.ones1, er_sb[:], start=True, stop=True)
    ab_sb = sb.tile([C, K], F32, tag="abc")
    nc.vector.tensor_copy(ab_sb[:], ab_ps[:])

    # Mix expert bank with UNNORMALIZED weights: wm = sum_k e[k] * wk[k]
    nc.vector.tensor_scalar_mul(wm_sb[:], wk_sb(0), ab_sb[:, 0:1])
    last = None
    for k in range(1, K):
        last = nc.vector.scalar_tensor_tensor(
            wm_sb[:], wk_sb(k), ab_sb[:, k:k + 1], wm_sb[:],
            op0=ALU.mult, op1=ALU.add)

    # Off-critical-path normalization: invs = broadcast(1/s) for the
    # epilogue scale; beta = (biasw.T @ e) * invs + c1 for the epilogue bias.
    rec_sb = sb.tile([1, 1], F32, tag="rec")
    nc.vector.reciprocal(rec_sb[:], s_sb[:])
    invs_ps = ps.tile([C, 1], F32, tag="tiny")
    nc.tensor.matmul(invs_ps[:], cc.ones1, rec_sb[:], start=True, stop=True)
    nc.vector.tensor_copy(invs_sb[:], invs_ps[:])
    lgc_ps = ps.tile([K, 1], F32, tag="tiny")
    nc.tensor.matmul(lgc_ps[:], cc.attk, prelu_sb[:], start=True, stop=True)
    ec_sb = sb.tile([K, 1], F32, tag="ecol")
    nc.scalar.activation(ec_sb[:], lgc_ps[:], AF.Exp)
    bm_ps = ps.tile([NF, 1], F32, tag="tiny")
    nc.tensor.matmul(bm_ps[:], cc.biasw, ec_sb[:], start=True, stop=True)
    nc.vector.tensor_scalar(beta_sb[:], bm_ps[:], invs_sb[:], cc.c1,
                            op0=ALU.mult, op1=ALU.add)
    return {"pool_a": ia, "pool_b": ib,  "mix_last": last}


def _emit_conv_chunks(nc, b, convps, xt_sb, wm_sb, beta_sb, invs_sb, y_sb,
                      y_dram, t_lo, t_hi):
    """9-tap conv chunks [t_lo, t_hi) as shifted fp16 matmuls + fused
    BN/bias/relu epilogue; output DMA'd out in pieces to shrink the tail."""
    xv = xt_sb.rearrange("p (h w) -> p h w", w=WP)
    for t in range(t_lo, t_hi):
        pc = convps.tile([NF, ROWS_PER_CHUNK * W], F32, tag="conv")
        for tap in range(TAPS):
            dy, dx = tap // 3, tap % 3
            r0 = ROWS_PER_CHUNK * t + dy
            rhs = xv[:, r0:r0 + ROWS_PER_CHUNK, dx:dx + W]
            nc.tensor.matmul(pc[:], wm_sb[:, NF * tap:NF * (tap + 1)], rhs,
                             start=(tap == 0), stop=(tap == TAPS - 1))
        nc.scalar.activation(y_sb[:, 512 * t:512 * (t + 1)], pc[:], AF.Relu,
                             bias=beta_sb[:], scale=invs_sb[:])
        if t == 3:
            nc.sync.dma_start(y_dram[b][:, :2048], y_sb[:, :2048])
        elif t == 5:
            nc.scalar.dma_start(y_dram[b][:, 2048:3072], y_sb[:, 2048:3072])
        elif t == 6:
            nc.sync.dma_start(y_dram[b][:, 3072:3584], y_sb[:, 3072:3584])
        elif t == 7:
            nc.scalar.dma_start(y_dram[b][:, 3584:3760], y_sb[:, 3584:3760])
            nc.sync.dma_start(y_dram[b][:, 3760:3936], y_sb[:, 3760:3936])
            nc.gpsimd.dma_start(y_dram[b][:, 3936:], y_sb[:, 3936:])


def _build_program():
    nc = bacc.Bacc("TRN2", target_bir_lowering=False, debug=False,
                   num_devices=NCORES)
    xt = nc.dram_tensor("xt", [BPC, C, NPAD], F16, kind="ExternalInput").ap()
    wk = nc.dram_tensor("wk", [C, K * TAPS * NF], F16,
                        kind="ExternalInput").ap()
    hotd = nc.dram_tensor("hot", [128, HOT_COLS], F32,
                          kind="ExternalInput").ap()
    coldd = nc.dram_tensor("cold", [128, COLD_COLS], F32,
                           kind="ExternalInput").ap()
    y = nc.dram_tensor("y", [BPC, NF, NPOS], F32, kind="ExternalOutput").ap()

    with tile.TileContext(nc) as tc:
        with (
            tc.tile_pool(name="const", bufs=1) as cpool,
            tc.tile_pool(name="xt", bufs=BPC) as xpool,
            tc.tile_pool(name="wm", bufs=BPC) as wmpool,
            tc.tile_pool(name="work", bufs=4) as sb,
            tc.tile_pool(name="ystage", bufs=2) as ypool,
            tc.tile_pool(name="convps", bufs=5, space="PSUM") as convps,
            tc.tile_pool(name="tinyps", bufs=2, space="PSUM") as ps,
        ):
            xt_sb = [xpool.tile([C, NPAD], F16, tag="xt", name=f"xt{b}")
                     for b in range(BPC)]
            # On-device constants: ones row + zeroed warm-up matmul source
            # (available immediately, no DMA).
            ones1_sb = cpool.tile([1, C], F32, tag="ones1")
            nc.gpsimd.memset(ones1_sb[:], 1.0)
            warm_src = cpool.tile([C, 512], F16, tag="warmsrc")
            nc.gpsimd.memset(warm_src[:], 0.0)

            # Startup loads spread across the three independent DMA paths
            # (each ring sustains only ~134 GB/s, so ring assignment decides
            # arrival time). Sample 0's input quarters land first (pool
            # reduces start on them immediately); the expert bank quarters
            # follow in the order the mixing chain consumes them; cold
            # consts (beta path) are only needed late.
            hot = cpool.tile([128, HOT_COLS], F32)
            cold = cpool.tile([128, COLD_COLS], F32)
            wk_all = cpool.tile([C, K * TAPS * NF], F16)
            WQ = TAPS * NF

            nc.sync.dma_start(hot[:], hotd)
            nc.scalar.dma_start(xt_sb[0][:, :HALF], xt[0][:, :HALF])
            nc.gpsimd.dma_start(xt_sb[0][:, HALF:], xt[0][:, HALF:])
            nc.sync.dma_start(wk_all[:, 0:WQ], wk[:, 0:WQ])
            nc.sync.dma_start(wk_all[:, 2 * WQ:3 * WQ], wk[:, 2 * WQ:3 * WQ])
            nc.scalar.dma_start(cold[:], coldd)
            nc.scalar.dma_start(wk_all[:, WQ:2 * WQ], wk[:, WQ:2 * WQ])
            nc.gpsimd.dma_start(wk_all[:, 3 * WQ:], wk[:, 3 * WQ:])
            cc = _Consts(hot[:], cold[:], ones1_sb[:])

            def wk_sb(k):
                return wk_all[:, k * WQ:(k + 1) * WQ]

            # Pre-load the ACT spline table set (relu+exp share one set).
            warm_sb = cpool.tile([1, 1], F32, tag="warm")
            nc.scalar.activation(warm_sb[:], ones1_sb[:, 0:1], AF.Exp)

            trash = cpool.tile([C, HALF], F16, tag="trash")

            wm_sb = [wmpool.tile([C, TAPS * NF], F16, tag="wm",
                                 name=f"wm{b}") for b in range(BPC)]
            beta_sb = [sb.tile([NF, 1], F32, tag="beta", name=f"beta{b}")
                       for b in range(BPC)]
            invs_sb = [sb.tile([NF, 1], F32, tag="invs", name=f"invs{b}")
                       for b in range(BPC)]
            y_sb = [ypool.tile([NF, NPOS], F32, tag="ystage", name=f"yst{b}")
                    for b in range(BPC)]

            # PE warm-up: fine-grained fp16 matmuls on the memset source so
            # the array stays busy (HAM at full clock) through the startup
            # window without inserting large delays into the tiny-matmul
            # routing chain interleaved with them.
            warm_ps = ps.tile([NF, 512], F32, tag="warmps", bufs=1)

            def pe_warm(n, cols=256):
                for _ in range(n):
                    nc.tensor.matmul(warm_ps[:, :cols], warm_src[:, 0:NF],
                                     warm_src[:, 0:cols], start=True,
                                     stop=True)

            def emit_next_xt(bn, prev):
                # Sample bn's input on the GPSIMD + scalar rings, gated on
                # sample bn-1's input being fully resident (its last pool
                # quarter-reduces) so transfers don't fight for HBM early.
                da = nc.gpsimd.dma_start(xt_sb[bn][:, :HALF],
                                         xt[bn][:, :HALF])
                db = nc.scalar.dma_start(xt_sb[bn][:, HALF:],
                                         xt[bn][:, HALF:])
                add_dep_helper(da.ins, prev["pool_a"].ins,
                               reason="stagger input DMA bandwidth")
                add_dep_helper(db.ins, prev["pool_b"].ins,
                               reason="stagger input DMA bandwidth")

            pe_warm(WARM1, cols=512)
            chains = [None] * BPC
            chains[0] = _emit_pool(nc, 0, sb, xt_sb[0][:], trash)
            _emit_chain(nc, 0, sb, ps, xt_sb[0][:], cc, wk_sb, wm_sb[0],
                        beta_sb[0], invs_sb[0], chains[0])
            emit_next_xt(1, chains[0])
            pe_warm(WARM2)
            # Per sample b: the NEXT sample's pool reduces go ahead of
            # conv_b's chunks (so the ACT-side reduce precedes conv_b's
            # epilogues in the scalar-engine queue); the rest of its routing
            # chain is emitted between chunks 2 and 3.
            for b in range(BPC):
                if b + 1 < BPC:
                    chains[b + 1] = _emit_pool(nc, b + 1, sb,
                                               xt_sb[b + 1][:], trash,
                                               act_only=(b + 1 == BPC - 1))
                    if not chains[b + 1]["act_only"]:
                        add_dep_helper(chains[b + 1]["pool_b"].ins,
                                       chains[b]["mix_last"].ins,
                                       reason="keep DVE reduce after prev mix")
                _emit_conv_chunks(nc, b, convps, xt_sb[b][:], wm_sb[b],
                                  beta_sb[b], invs_sb[b], y_sb[b], y, 0, 3)
                if b + 1 < BPC:
                    _emit_chain(nc, b + 1, sb, ps, xt_sb[b + 1][:], cc, wk_sb,
                                wm_sb[b + 1], beta_sb[b + 1], invs_sb[b + 1],
                                chains[b + 1])
                    if b + 2 < BPC:
                        emit_next_xt(b + 2, chains[b + 1])
                _emit_conv_chunks(nc, b, convps, xt_sb[b][:], wm_sb[b],
                                  beta_sb[b], invs_sb[b], y_sb[b], y, 3,
                                  NCHUNK)

    nc.compile()
    return nc


_PROGRAM = None


def _get_program():
    global _PROGRAM
    if _PROGRAM is None:
        _PROGRAM = _build_program()
    return _PROGRAM


def _prepare_host_inputs(x, reduction_kernel, attention_kernel, conv_kernels,
                         bias, bn_scale, bn_bias, bn_mean, bn_var):
    f = np.float32
    # Channel-major zero-padded fp16 input: [B, C, 66, 66]
    xt = np.zeros((B, C, HP, WP), dtype=np.float16)
    xt[:, :, 1:H + 1, 1:W + 1] = x.transpose(0, 3, 1, 2)
    xt = xt.reshape(B, C, NPAD)

    inv = (bn_scale / np.sqrt(bn_var + np.float32(1e-5))).astype(f)
    # Expert bank [C, k*tap*F] fp16, BN scale folded into F.
    wkh = (conv_kernels.transpose(0, 3, 1, 2, 4) * inv).astype(f)
    wkh = wkh.reshape(K, C, TAPS * NF).transpose(1, 0, 2).reshape(
        C, K * TAPS * NF)
    wkh = np.ascontiguousarray(wkh, dtype=np.float16)

    hot = _pack_hot(
        red=(reduction_kernel / np.float32(NPOS)).astype(f),
        attk=(attention_kernel / np.float32(30.0)).astype(f),
    )
    cold = _pack_cold(
        biasw=(bias * inv).astype(f),
        c1=(bn_bias - bn_mean * inv).astype(f),
    )

    in_maps = []
    for cix in range(NCORES):
        in_maps.append({
            "xt": np.ascontiguousarray(xt[cix * BPC:(cix + 1) * BPC]),
            "wk": wkh,
            "hot": hot,
            "cold": cold,
        })
    return in_maps


def kernel(x, reduction_kernel, attention_kernel, conv_kernels, bias, bn_scale,
           bn_bias, bn_mean, bn_var, _trace=False):
    nc = _get_program()
    in_maps = _prepare_host_inputs(
        np.asarray(x, dtype=np.float32), np.asarray(reduction_kernel, np.float32),
        np.asarray(attention_kernel, np.float32),
        np.asarray(conv_kernels, np.float32), np.asarray(bias, np.float32),
        np.asarray(bn_scale, np.float32), np.asarray(bn_bias, np.float32),
        np.asarray(bn_mean, np.float32), np.asarray(bn_var, np.float32))
    if _trace:
        _ensure_ntff_hook()
    res = run_bass_kernel_spmd(nc, in_maps, core_ids=list(range(NCORES)),
                               trace=_trace)
    yt = np.concatenate([res.results[cix]["y"] for cix in range(NCORES)],
                        axis=0)  # [B, F, 4096]
    out = yt.reshape(B, NF, H, W).transpose(0, 2, 3, 1)
    out = np.ascontiguousarray(out, dtype=np.float32)
    if _trace:
        return out, res
    return out
